# revision 6
# baseline (speedup 1.0000x reference)
"""AddContextFrames distributed Trainium2 kernel.

out[0, w*80+f, t] = signal[0, f, t + w - 9]  (zero outside), w in 0..18.

Strategy: shard the time axis across 8 NeuronCores. Each core receives a
zero-padded input shard (80, 4096+18) that already includes the halo, so no
inter-core communication is needed. On-core: one DMA load into SBUF, then 19
shifted-window DMA stores into the (1520, 4096) output shard.
"""

import numpy as np

import concourse.bass as bass
import concourse.mybir as mybir
from concourse.bass_utils import run_bass_kernel_spmd

N_CORES = 8
N_CONTEXT = 9
WINDOW = 2 * N_CONTEXT + 1  # 19
FEATS = 80
STEPS = 32768
SHARD = STEPS // N_CORES    # 4096
HALO = 2 * N_CONTEXT        # 18
IN_W = SHARD + HALO         # 4114
OUT_CH = WINDOW * FEATS     # 1520

_nc_cache = None


# Port-balanced SBUF layout: sub-row s = f*8 + b (f feature, b 512-step time
# block) lives at partition s % 128, region r = s // 128 (5 regions), holding
# x[f, b*512 : b*512+530].  All 128 partitions (16 SBUF ports) carry equal
# load, unlike the naive 80-partition layout (62.5% of port bandwidth).
NB = 8               # time sub-blocks per feature
TB = SHARD // NB     # 512
SUBW = TB + HALO     # 530
NR = (FEATS * NB) // 128  # 5 regions per partition
PITCH = 536          # sub-row pitch in elements (32B aligned)


def build_nc() -> bass.Bass:
    from concourse.ap import AP

    nc = bass.Bass()
    x = nc.declare_dram_parameter(
        "signal", [FEATS, IN_W], mybir.dt.float32, isOutput=False
    )
    out = nc.declare_dram_parameter(
        "out", [OUT_CH, SHARD], mybir.dt.float32, isOutput=True
    )
    # Split issue across both HWDGE sequencers (SP + ACT): descriptor
    # generation is ~5.3 ns/descriptor serial per sequencer, which on one
    # sequencer (~69 us for 12800 descriptors) co-limits with the ~68 us of
    # HBM time.  Two sequencers halve it and it vanishes under the HBM time.
    # Store (w, r) depends only on the two load halves of region r.
    def store_ap(w, r):
        return AP(out, w * FEATS * SHARD + r * 128 * TB, [[TB, 128], [1, TB]])

    def load_ap(r, h):
        return AP(
            x, (r * 16 + h * 8) * IN_W, [[IN_W, 8], [TB, 8], [1, SUBW]]
        )

    # Alternate stores between the two sequencers.
    jobs = [(r, w) for r in range(NR) for w in range(WINDOW)]
    sync_jobs = [j for i, j in enumerate(jobs) if i % 2 == 1]
    scalar_jobs = [j for i, j in enumerate(jobs) if i % 2 == 0]

    from contextlib import ExitStack

    with ExitStack() as stack:
        tile = stack.enter_context(
            nc.sbuf_tensor([128, NR, PITCH], mybir.dt.float32)
        )
        load_sems = [
            stack.enter_context(nc.semaphore(f"ld{r}")) for r in range(NR)
        ]
        ss_a = stack.enter_context(nc.semaphore("ss_a"))
        ss_b = stack.enter_context(nc.semaphore("ss_b"))
        block = stack.enter_context(nc.Block())

        @block.sync
        def _(sync):
            for r in range(NR):
                sync.dma_start(
                    out=tile[0:64, r, 0:SUBW], in_=load_ap(r, 0)
                ).then_inc(load_sems[r], 16)
            last_r = -1
            for r, w in sync_jobs:
                if r != last_r:
                    sync.wait_ge(load_sems[r], 32)
                    last_r = r
                sync.dma_start(
                    out=store_ap(w, r), in_=tile[:, r, w : w + TB]
                ).then_inc(ss_a, 16)
            sync.wait_ge(ss_a, 16 * len(sync_jobs))

        @block.scalar
        def _(scalar):
            for r in range(NR):
                scalar.dma_start(
                    out=tile[64:128, r, 0:SUBW], in_=load_ap(r, 1)
                ).then_inc(load_sems[r], 16)
            last_r = -1
            for r, w in scalar_jobs:
                if r != last_r:
                    scalar.wait_ge(load_sems[r], 32)
                    last_r = r
                scalar.dma_start(
                    out=store_ap(w, r), in_=tile[:, r, w : w + TB]
                ).then_inc(ss_b, 16)
            scalar.wait_ge(ss_b, 16 * len(scalar_jobs))

    return nc


def _install_ntff_hook():
    """The image lacks antenv.axon_hooks; synthesize it so trace=True works."""
    import sys, types

    if "antenv.axon_hooks" in sys.modules:
        return
    try:
        from trn_agent_boot.trn_boot import _ntff_profile_via_ctypes

        mod = types.ModuleType("antenv.axon_hooks")
        _state = {"hook": _ntff_profile_via_ctypes("/opt/axon/libaxon_pjrt.so")}
        mod.get_axon_ntff_profile_hook = lambda: _state["hook"]
        mod.set_axon_ntff_profile_hook = lambda h: _state.__setitem__("hook", h)
        sys.modules["antenv.axon_hooks"] = mod
        import antenv

        antenv.axon_hooks = mod
    except Exception:
        pass


def run(signal: np.ndarray, trace: bool = False):
    """signal: (1, 80, 32768) f32 -> ((1, 1520, 32768) f32, exec_time_ns|None)"""
    global _nc_cache
    if trace:
        _install_ntff_hook()
    signal = np.asarray(signal, dtype=np.float32)
    xp = np.zeros((FEATS, STEPS + HALO), np.float32)
    xp[:, N_CONTEXT : N_CONTEXT + STEPS] = signal[0]
    in_maps = [
        {"signal": np.ascontiguousarray(xp[:, i * SHARD : i * SHARD + IN_W])}
        for i in range(N_CORES)
    ]
    if _nc_cache is None:
        _nc_cache = build_nc()
    res = run_bass_kernel_spmd(
        _nc_cache, in_maps, core_ids=list(range(N_CORES)), trace=trace
    )
    out = np.empty((1, OUT_CH, STEPS), np.float32)
    for i in range(N_CORES):
        out[0, :, i * SHARD : (i + 1) * SHARD] = np.asarray(res.results[i]["out"])
    return out, res


def kernel(signal: np.ndarray) -> np.ndarray:
    out, _ = run(signal, trace=False)
    return out


# revision 10
# speedup vs baseline: 1.0908x; 1.0908x over previous
"""AddContextFrames distributed Trainium2 kernel.

out[0, w*80+f, t] = signal[0, f, t + w - 9]  (zero outside), w in 0..18.

Strategy: shard the time axis across 8 NeuronCores. Each core receives a
zero-padded input shard (80, 4096+18) that already includes the halo, so no
inter-core communication is needed. On-core: one DMA load into SBUF, then 19
shifted-window DMA stores into the (1520, 4096) output shard.
"""

import numpy as np

import concourse.bass as bass
import concourse.mybir as mybir
from concourse.bass_utils import run_bass_kernel_spmd

N_CORES = 8
N_CONTEXT = 9
WINDOW = 2 * N_CONTEXT + 1  # 19
FEATS = 80
STEPS = 32768
SHARD = STEPS // N_CORES    # 4096
HALO = 2 * N_CONTEXT        # 18
IN_W = SHARD + HALO         # 4114
OUT_CH = WINDOW * FEATS     # 1520

_nc_cache = None


# Port-balanced SBUF layout with large DMA descriptors.  Each feature row is
# split into 2 time blocks of 2048 (sub-row s = 2f + b holds
# x[f, b*2048 : b*2048 + 2066], incl. 18-elem halo).  Region 0: sub-rows
# 0..127 on partition s.  Region 1: sub-rows 128..159 on partitions 4j
# (j = s - 128) — exactly 2 per SBUF AXI port, so all 16 ports carry equal
# load (10 sub-rows/port/window).  Stores issue in 1024-elem halves → 4 KB
# descriptors, few enough (~6400 total) that single-ring HWDGE descriptor
# generation (~5.3 ns/desc) stays well under the ~68 us HBM time.
TB = 2048            # time block
HB = TB // 2         # 1024 store/load half
SUBW = TB + HALO     # 2066
SUBH = HB + HALO     # 1042 (half load width)
PITCH = 2072         # sub-row pitch in elements (32B aligned)


def build_nc() -> bass.Bass:
    from concourse.ap import AP

    nc = bass.Bass()
    x = nc.declare_dram_parameter(
        "signal", [FEATS, IN_W], mybir.dt.float32, isOutput=False
    )
    out = nc.declare_dram_parameter(
        "out", [OUT_CH, SHARD], mybir.dt.float32, isOutput=True
    )
    from contextlib import ExitStack

    with ExitStack() as stack:
        tile = stack.enter_context(
            nc.sbuf_tensor([128, 2, PITCH], mybir.dt.float32)
        )

        ld = [
            [stack.enter_context(nc.semaphore(f"ld{r}{h}")) for h in (0, 1)]
            for r in (0, 1)
        ]
        ss = stack.enter_context(nc.semaphore("ss"))
        block = stack.enter_context(nc.Block())

        @block.sync
        def _(sync):
            # loads: disjoint sub-row column halves [0:1042) and [1042:2066).
            # h=1 stores read [w+1024, w+2048) which spans both halves; the
            # issue order (h0 stores, with their waits, precede h1) covers it.
            for h, c0, cw in ((0, 0, SUBH), (1, SUBH, HB)):
                sync.dma_start(
                    out=tile[:, 0, c0 : c0 + cw],
                    in_=AP(x, c0, [[IN_W, 64], [TB, 2], [1, cw]]),
                ).then_inc(ld[0][h], 16)
                sync.dma_start(
                    out=tile[0:128:4, 1, c0 : c0 + cw],
                    in_=AP(
                        x, 64 * IN_W + c0, [[IN_W, 16], [TB, 2], [1, cw]]
                    ),
                ).then_inc(ld[1][h], 16)
            n = 0
            for h in (0, 1):
                for r in (0, 1):
                    sync.wait_ge(ld[r][h], 16)
                    for w in range(WINDOW):
                        if r == 0:
                            sb = tile[:, 0, w + h * HB : w + h * HB + HB]
                            dst = AP(
                                out,
                                w * FEATS * SHARD + h * HB,
                                [[TB, 128], [1, HB]],
                            )
                        else:
                            sb = tile[0:128:4, 1, w + h * HB : w + h * HB + HB]
                            dst = AP(
                                out,
                                w * FEATS * SHARD + 128 * TB + h * HB,
                                [[TB, 32], [1, HB]],
                            )
                        sync.dma_start(out=dst, in_=sb).then_inc(ss, 16)
                        n += 1
            sync.wait_ge(ss, 16 * n)

    return nc


def _install_ntff_hook():
    """The image lacks antenv.axon_hooks; synthesize it so trace=True works."""
    import sys, types

    if "antenv.axon_hooks" in sys.modules:
        return
    try:
        from trn_agent_boot.trn_boot import _ntff_profile_via_ctypes

        mod = types.ModuleType("antenv.axon_hooks")
        _state = {"hook": _ntff_profile_via_ctypes("/opt/axon/libaxon_pjrt.so")}
        mod.get_axon_ntff_profile_hook = lambda: _state["hook"]
        mod.set_axon_ntff_profile_hook = lambda h: _state.__setitem__("hook", h)
        sys.modules["antenv.axon_hooks"] = mod
        import antenv

        antenv.axon_hooks = mod
    except Exception:
        pass


def run(signal: np.ndarray, trace: bool = False):
    """signal: (1, 80, 32768) f32 -> ((1, 1520, 32768) f32, exec_time_ns|None)"""
    global _nc_cache
    if trace:
        _install_ntff_hook()
    signal = np.asarray(signal, dtype=np.float32)
    xp = np.zeros((FEATS, STEPS + HALO), np.float32)
    xp[:, N_CONTEXT : N_CONTEXT + STEPS] = signal[0]
    in_maps = [
        {"signal": np.ascontiguousarray(xp[:, i * SHARD : i * SHARD + IN_W])}
        for i in range(N_CORES)
    ]
    if _nc_cache is None:
        _nc_cache = build_nc()
    res = run_bass_kernel_spmd(
        _nc_cache, in_maps, core_ids=list(range(N_CORES)), trace=trace
    )
    out = np.empty((1, OUT_CH, STEPS), np.float32)
    for i in range(N_CORES):
        out[0, :, i * SHARD : (i + 1) * SHARD] = np.asarray(res.results[i]["out"])
    return out, res


def kernel(signal: np.ndarray) -> np.ndarray:
    out, _ = run(signal, trace=False)
    return out
